# revision 46
# baseline (speedup 1.0000x reference)
"""Trainium2 Bass kernel for DynamicToeplitzMultihead.

Math: the reference's ortho-normalized FFT Toeplitz convolution is exactly
    out[b, h] = T_h @ x[b, h],   T_h[t, s] = a_h[(t - s) mod 2n]
where a_h (length 2n = 4096) is produced by a tiny MLP (DynamicPosBias) on
the 2047 relative positions plus a log-sigmoid decay term.  The MLP is
O(n * 16) work — computed on host — while the 2048x2048xE matmuls per
(batch, head) run on the tensor engines.

Sharding: head-parallel across the 8 cores.  Core h gets x[:, h]
([16, 2048, 64]) plus the 31 distinct 128x128 Toeplitz tiles of T_h
(tile-level diagonal-constant structure), and computes out[:, h] with
512 accumulating fp32r matmuls (free dim 512 = 8 batches x 64 channels).

fp32r notes: fp32r is fp32 rounded to an 11-bit mantissa (TF32-like),
which runs matmuls at full PE rate (1 cycle/row at free dim >= 256) vs
4 cycles/row for fp32.  Host arrays are pre-rounded to the fp32r grid.
All matmul inputs are produced by DVE copies so that every matmul carries
at most ONE semaphore wait (the fp32r self-loading LDW ISA slot cannot
hold more).
"""

import sys

import numpy as np

for _p in ("/opt/trn_rl_repo",):
    if _p not in sys.path:
        sys.path.append(_p)

B, H, N, E = 16, 8, 2048, 64
NT = N // 128          # 16 tiles of 128 along the sequence axis
ND = 2 * NT - 1        # 31 distinct Toeplitz tiles per head
BG = 2                 # batch groups of 8 (8 * 64 = 512 free dim)
BPG = B // BG          # batches per group

_PROGRAM = None


def _ln(x, g, b):
    m = x.mean(-1, keepdims=True)
    v = x.var(-1, keepdims=True)
    return (x - m) / np.sqrt(v + 1e-5) * g + b


def _compute_a(gamma, w0, b0, ln1_g, ln1_b, w1, b1, ln2_g, ln2_b, w2, b2,
               ln3_g, ln3_b, w3, b3):
    """Toeplitz coefficients a [H, 2N] (float64), mirroring the reference."""
    d = np.float64
    w0, b0, w1, b1, w2, b2, w3, b3 = (t.astype(d) for t in (w0, b0, w1, b1, w2, b2, w3, b3))
    ln1_g, ln1_b, ln2_g, ln2_b, ln3_g, ln3_b = (
        t.astype(d) for t in (ln1_g, ln1_b, ln2_g, ln2_b, ln3_g, ln3_b))
    gamma = gamma.astype(d)

    def dpb(t):
        h = t @ w0 + b0
        h = np.maximum(_ln(h, ln1_g, ln1_b), 0) @ w1 + b1
        h = np.maximum(_ln(h, ln2_g, ln2_b), 0) @ w2 + b2
        return np.maximum(_ln(h, ln3_g, ln3_b), 0) @ w3 + b3

    pos_t = np.arange(1, N, dtype=d)[:, None]
    pd = dpb(pos_t).T                                  # [H, N-1]
    zero_dpb = dpb(np.zeros((1, 1), d)).T              # [H, 1]
    coef = np.arange(1, N, dtype=d)[None]
    glog = np.log(1.0 / (1.0 + np.exp(-gamma))) * coef  # [1, N-1]
    pos = glog + pd
    neg = glog[:, ::-1] + pd
    return np.exp(np.clip(
        np.concatenate([zero_dpb, pos, zero_dpb, neg], axis=-1), -60.0, 30.0))


def _round_fp32r(arr):
    """Round float32 to the fp32r grid (11-bit mantissa, RNE) like HW does."""
    u = np.ascontiguousarray(arr, np.float32).view(np.uint32)
    r = (u + np.uint32(0x7FF) + ((u >> np.uint32(12)) & np.uint32(1))) & np.uint32(0xFFFFF000)
    return r.view(np.float32)


def _toeplitz_tiles(a_h):
    """lhsT tiles for one head: [128 j, ND * 128] with
    tt[j, d*128 + i] = a_h[(128*(d - 15) + i - j) mod 2N]."""
    j = np.arange(128)[:, None, None]
    dd = np.arange(ND)[None, :, None] - (NT - 1)
    i = np.arange(128)[None, None, :]
    idx = (128 * dd + i - j) % (2 * N)
    return _round_fp32r(a_h[idx].reshape(128, ND * 128).astype(np.float32))


def _build_program_raw():
    """Hand-scheduled raw-bass version: minimal semaphores (Tile's per-matmul
    sem updates cost ~26ns each; here only group-boundary matmuls carry sync),
    no Tile preamble/drain."""
    import concourse.bacc as bacc
    import concourse.mybir as mybir
    from contextlib import ExitStack

    f32 = mybir.dt.float32
    f32r = mybir.dt.float32r

    nc = bacc.Bacc("TRN2", target_bir_lowering=False, debug=False, num_devices=H)
    xs = nc.declare_dram_parameter("xs", [NT, 128, BG, BPG * E], f32r, isOutput=False)
    tt = nc.declare_dram_parameter("tt", [128, ND * 128], f32r, isOutput=False)
    out = nc.declare_dram_parameter("out", [NT, 128, BG, BPG * E], f32, isOutput=True)

    NPS = 8                       # psum banks (phase A holds all 8 groups)
    NOT = 8                       # output staging tiles in rotation
    groups = [(bg, ti) for bg in range(BG) for ti in range(NT)]
    t_chunks = ((0, 256), (256, 1024), (1024, 2048), (2048, ND * 128))

    def chunk_of(d):
        for c, (lo, hi) in enumerate(t_chunks):
            if d * 128 < hi:
                return c
        raise AssertionError

    with ExitStack() as ctx:
        tmega = ctx.enter_context(nc.sbuf_tensor("tmega", [128, ND * 128], f32r))
        # per-(bg, si) tiles: phase A only needs bg=0's 4.2MB, so bg=1
        # streams later, during the dense phase B.
        xt = {(bg, si): ctx.enter_context(
                  nc.sbuf_tensor(f"xt{bg}_{si}", [128, BPG * E], f32r))
              for bg in range(BG) for si in range(NT)}
        ot = [ctx.enter_context(nc.sbuf_tensor(f"ot{i}", [128, BPG * E], f32))
              for i in range(NOT)]
        ps = [ctx.enter_context(nc.psum_tensor(f"ps{i}", [128, BPG * E], f32))
              for i in range(NPS)]
        tsem = [ctx.enter_context(nc.semaphore(f"tsem{c}"))
                for c in range(len(t_chunks))]
        xsem = {(bg, si): ctx.enter_context(nc.semaphore(f"xsem{bg}_{si}"))
                for bg in range(BG) for si in range(NT)}
        osem = [ctx.enter_context(nc.semaphore(f"osem{g}"))
                for g in range(len(groups))]
        pe_sem = ctx.enter_context(nc.semaphore("pe_sem"))
        dve = ctx.enter_context(nc.semaphore("dve"))

        def x_dma(eng, bg, si):
            eng.dma_start(out=xt[bg, si][:],
                          in_=xs[si, :, bg, :]).then_inc(xsem[bg, si], 16)

        with nc.Block() as block:

            @block.sync
            def _(sync):
                for si in (15, 14, 13, 12, 11, 9, 8, 7, 5, 3):
                    x_dma(sync, 0, si)
                for si in range(NT - 1, -1, -1):
                    x_dma(sync, 1, si)

            @block.scalar
            def _(act):
                # Chunks + the rest of bg0, ordered by phase-A need time
                # against this ring's ~185GB/s delivery rate.
                act_seq = ["c0", "c1", "c2", 10, "c3", 6, 4, 2, 1, 0]
                for item in act_seq:
                    if isinstance(item, str):
                        c = int(item[1])
                        lo, hi = t_chunks[c]
                        act.dma_start(out=tmega[:, lo:hi],
                                      in_=tt[:, lo:hi]).then_inc(tsem[c], 16)
                    else:
                        x_dma(act, 0, item)
                ng = len(groups)
                for g, (bg, ti) in enumerate(groups):
                    if g < ng - 1:
                        act.wait_ge(dve, g + 1)
                        act.dma_start(out=out[ti, :, bg, :],
                                      in_=ot[g % NOT][:]).then_inc(osem[g], 16)
                    else:
                        # last group: 4 column-chunks to drain the tail faster
                        for k in range(4):
                            act.wait_ge(dve, g + 1 + k)
                            act.dma_start(
                                out=out[ti, :, bg, k * 128:(k + 1) * 128],
                                in_=ot[g % NOT][:, k * 128:(k + 1) * 128],
                            ).then_inc(osem[g], 16)
                # DVE's stream already implies osem[g] fired for g <= ng-1-NOT
                # (copy g+NOT waited on it); only the last NOT need explicit waits.
                for g in range(ng - NOT, ng - 1):
                    act.wait_ge(osem[g], 16)
                act.wait_ge(osem[ng - 1], 64)

            @block.vector
            def _(vec):
                ng = len(groups)
                for g in range(ng):
                    vec.wait_ge(pe_sem, g + 1)
                    if g >= NOT:
                        vec.wait_ge(osem[g - NOT], 16)
                    if g < ng - 1:
                        vec.tensor_copy(ot[g % NOT][:], ps[g % NPS][:]).then_inc(dve, 1)
                    else:
                        for k in range(4):
                            vec.tensor_copy(
                                ot[g % NOT][:, k * 128:(k + 1) * 128],
                                ps[g % NPS][:, k * 128:(k + 1) * 128],
                            ).then_inc(dve, 1)

            @block.tensor
            def _(pe):
                cur_chunk = -1

                def emit_mm(g, bg, ti, si):
                    nonlocal cur_chunk
                    d = ti - si + NT - 1
                    c = chunk_of(d)
                    if c > cur_chunk:
                        cur_chunk = c
                        pe.wait_ge(tsem[c], 16)
                    mm = pe.matmul(
                        ps[g % NPS][:],
                        tmega[:, d * 128:(d + 1) * 128],
                        xt[bg, si][:],
                        start=(si == NT - 1),
                        stop=(si == 0),
                    )
                    if si == 0:
                        mm.then_inc(pe_sem, 1)

                # Phase A: 8 groups (bg=0, ti=0..7) interleaved across all 8
                # psum banks, consuming x tiles strictly in arrival order —
                # 8 matmuls (~1.8us) of work per arriving tile keeps the PE
                # ahead of the DMA stream from the first tile on.
                for si in range(NT - 1, -1, -1):
                    pe.wait_ge(xsem[0, si], 16)
                    for g in range(NPS):
                        emit_mm(g, 0, g, si)

                # Phase B: remaining groups, dense (bg=0 resident; bg=1
                # tiles streamed in long before group 16 needs them).
                seen_x = set()
                for g in range(NPS, len(groups)):
                    bg, ti = groups[g]
                    for si in range(NT - 1, -1, -1):
                        if si == NT - 1:
                            pe.wait_ge(dve, g - NPS + 1)
                        if bg == 1 and si not in seen_x:
                            seen_x.add(si)
                            pe.wait_ge(xsem[1, si], 16)
                        emit_mm(g, bg, ti, si)

    nc.compile()
    return nc


def _build_program():
    import concourse.bass as bass
    import concourse.bacc as bacc
    import concourse.mybir as mybir
    import concourse.tile as tile
    from contextlib import ExitStack

    f32 = mybir.dt.float32
    f32r = mybir.dt.float32r

    nc = bacc.Bacc("TRN2", target_bir_lowering=False, debug=False, num_devices=H)
    # x / out live in tile layout [si, p, bg, b*e] so every DMA moves fully
    # contiguous >=512B runs (host does the transpose once).
    xs = nc.declare_dram_parameter("xs", [NT, 128, BG, BPG * E], f32r, isOutput=False)
    tt = nc.declare_dram_parameter("tt", [128, ND * 128], f32r, isOutput=False)
    out = nc.declare_dram_parameter("out", [NT, 128, BG, BPG * E], f32, isOutput=True)

    with tile.TileContext(nc) as tc, ExitStack() as ctx:
        tp = ctx.enter_context(tc.tile_pool(name="tp", bufs=1))
        xp = ctx.enter_context(tc.tile_pool(name="xp", bufs=BG * NT))
        op = ctx.enter_context(tc.tile_pool(name="op", bufs=6))
        pp = ctx.enter_context(tc.tile_pool(name="pp", bufs=6, space="PSUM"))
        wp = ctx.enter_context(tc.tile_pool(name="wp", bufs=1, space="PSUM"))

        # T tiles (host pre-rounded fp32r): DMA on the ACT HWDGE ring so the
        # x DMAs on the SP ring start at t=0.  Chunked so the first matmuls
        # (group ti=0 consumes d ascending) start after ~0.5MB.
        # T tiles on the ACT ring (chunked; group ti=0 consumes d ascending),
        # x tiles on the SP ring, si descending to match in-group consumption.
        tmega = tp.tile([128, ND * 128], f32r)
        for lo, hi in ((0, 256), (256, 1024), (1024, 2048), (2048, ND * 128)):
            nc.scalar.dma_start(out=tmega[:, lo:hi], in_=tt[:, lo:hi])

        xtiles = {}
        for bg in range(BG):
            for si in range(NT - 1, -1, -1):
                xt = xp.tile([128, BPG * E], f32r)
                nc.sync.dma_start(out=xt[:], in_=xs[si, :, bg, :])
                xtiles[bg, si] = xt

        # All bg=0 groups first: once bg=0's 16 x tiles are resident (~11us)
        # the PE has 16 dense groups to chew while bg=1 tiles stream in.
        # si descending inside a group puts the group's only
        # never-before-seen weight slice (d = 15 + ti) on the last
        # (non-start) matmul, keeping matmul waits minimal.
        for bg in range(BG):
            for ti in range(NT):
                ps = pp.tile([128, BPG * E], f32)
                for si in range(NT - 1, -1, -1):
                    d = ti - si + NT - 1
                    nc.tensor.matmul(
                        ps[:],
                        tmega[:, d * 128:(d + 1) * 128],
                        xtiles[bg, si][:],
                        start=(si == NT - 1),
                        stop=(si == 0),
                    )
                ot = op.tile([128, BPG * E], f32)
                last = (bg == BG - 1 and ti == NT - 1)
                # Last group: chunk the copy+DMA so the store pipeline drains
                # faster after the final matmul.
                for lo, hi in (((0, 128), (128, 256), (256, 384), (384, 512))
                               if last else ((0, BPG * E),)):
                    nc.vector.tensor_copy(ot[:, lo:hi], ps[:, lo:hi])
                    nc.scalar.dma_start(out=out[ti, :, bg, lo:hi],
                                        in_=ot[:, lo:hi])
    nc.compile()
    return nc


def _shard_x(x_h):
    """[B, N, E] -> tile layout [NT, 128, BG, BPG*E], fp32r-rounded."""
    v = x_h.reshape(BG, BPG, NT, 128, E).transpose(2, 3, 0, 1, 4)
    return _round_fp32r(v.reshape(NT, 128, BG, BPG * E))


def _unshard_out(o_h):
    """tile layout [NT, 128, BG, BPG*E] -> [B, N, E]."""
    v = o_h.reshape(NT, 128, BG, BPG, E).transpose(2, 3, 0, 1, 4)
    return v.reshape(B, N, E)


def kernel(**inputs):
    global _PROGRAM
    inputs = {k: np.asarray(v) for k, v in inputs.items()}
    x = np.ascontiguousarray(inputs.pop("x").astype(np.float32, copy=False))

    a = _compute_a(**inputs)                       # [H, 2N] float64

    if _PROGRAM is None:
        import os
        if os.environ.get("TOEPLITZ_TILE_KERNEL"):
            _PROGRAM = _build_program()
        else:
            _PROGRAM = _build_program_raw()
    nc = _PROGRAM

    from concourse.bass_utils import run_bass_kernel_spmd

    in_maps = [
        {
            "xs": _shard_x(x[:, h]),
            "tt": _toeplitz_tiles(a[h]),
        }
        for h in range(H)
    ]
    res = run_bass_kernel_spmd(nc, in_maps, list(range(H)))
    return np.stack([_unshard_out(res.results[h]["out"]) for h in range(H)], axis=1)
